# revision 74
# baseline (speedup 1.0000x reference)
"""CrossAttention kernel for 8 Trainium2 NeuronCores.

Reference computation (per batch element b):
    q = ts[b] @ q_w.T + q_b          # [512, 1024]
    k = llm[b] @ k_w.T + k_b         # [2048, 1024]
    v = llm[b] @ v_w.T + v_b         # [2048, 1024]
    per head h (16 heads x 64 dims):
        scores = q_h @ k_h.T / 8     # [512, 2048]
        attn = softmax(scores, -1)
        ctx_h = attn @ v_h           # [512, 64]
    out = ctx @ o_w.T + o_b          # [512, 1024]

Sharding: data-parallel over batch (B=8 -> one element per core), no
collectives.

Per-core structure (PE matmul cost on TRN2 is proportional ONLY to the
output free-dim size, so every matmul is oriented to keep the moving
dim minimal):

  QT[j, p]   = q_w @ ts.T  + q_b      (feature-major, bias per-partition)
  KT[j, s]   = k_w @ llm.T + k_b
  V'[s, j']  = llm @ v_w.T + v_b      (j' = 16 heads x 65 cols; col 64 of
                                       each head block is ones -> denom)
  scoresT_h[s, p] = KT_h.T @ QT_h     (fp8e4m3 DoubleRow matmul, 0.5
                                       cycles/row; qt/kt cast to fp8 by the
                                       projection bias-add, then repacked to
                                       the [32, 2, *] DoubleRow layout by
                                       SBUF->SBUF DMAs on the idle
                                       Pool/SWDGE queue; adds ~1e-2 rel err,
                                       gate is 2e-2)
  expT = exp(scoresT / 8)             (no max subtraction: |scores/8| < ~3)
  ctx_h[p-chunk, 0:65] = expT_h(:,pc).T @ V'_h   accumulated over s-tiles
                                      (out free = 65, NOT 512; col 64 =
                                       softmax denominator)
  ctx_nat[p, d] = ctx * (1/denom)     (per-partition scalar mul on DVE)
  cxT[d, p]  = PE-transpose(ctx_nat)  (identity matmul, 128x128 blocks)
  out[p, j]  = cxT.T @ o_wT + o_b     (d<7 partials pre-accumulated during
                                       the last head pair with the bias
                                       folded in; the tail is just the d=7
                                       matmul, an identity-matmul that
                                       accumulates the bf16 partial into
                                       PSUM, an Act-engine copy, and DMA)

Emission is software-pipelined: ctx matmuls lag scores/exp by one
stage so PE never waits in-order on the Act engine; K-projection
groups for head-pair p+1 are spread through pair p's stages as PE
filler; O-projection partials fill pair 7.

All matmuls bf16 inputs / fp32 PSUM accumulate.  Host does layout-only
prep (transpose, bf16 cast, bias broadcast).
"""
import numpy as np
import ml_dtypes

D = 1024          # d_model
P = 512           # ts sequence length
S = 2048          # llm sequence length
H = 16            # heads
DH = 64           # head dim
NCORES = 8
NDT = D // 128    # 8 d-tiles
NST = S // 128    # 16 s-tiles
NPT = P // 128    # 4 p-tiles
NPAIR = H // 2    # 8 head pairs

_BF16 = ml_dtypes.bfloat16

_cached_nc = None


def _build_nc():
    import concourse.tile as tile
    from concourse import bacc, mybir

    f32 = mybir.dt.float32
    bf16 = mybir.dt.bfloat16

    nc = bacc.Bacc("TRN2", target_bir_lowering=False, debug=False,
                   num_devices=NCORES)

    tsT = nc.declare_dram_parameter("tsT", [D, P], bf16, isOutput=False)
    llmT = nc.declare_dram_parameter("llmT", [D, S], bf16, isOutput=False)
    qwT = nc.declare_dram_parameter("qwT", [D, D], bf16, isOutput=False)
    kwT = nc.declare_dram_parameter("kwT", [D, D], bf16, isOutput=False)
    vwT = nc.declare_dram_parameter("vwT", [D, D], bf16, isOutput=False)
    owT = nc.declare_dram_parameter("owT", [D, D], bf16, isOutput=False)
    qkb = nc.declare_dram_parameter("qkb", [128, 2 * NDT], f32, isOutput=False)
    vbb = nc.declare_dram_parameter("vbb", [128, D], bf16, isOutput=False)
    obb = nc.declare_dram_parameter("obb", [128, D], bf16, isOutput=False)
    out = nc.declare_dram_parameter("out", [P, D], bf16, isOutput=True)

    with tile.TileContext(nc) as tc:
        _emit(tc, nc, tile, mybir, f32, bf16,
              tsT, llmT, qwT, kwT, vwT, owT, qkb, vbb, obb, out)
    nc.compile()
    return nc


def _emit(tc, nc, tile, mybir, f32, bf16,
          tsT, llmT, qwT, kwT, vwT, owT, qkb, vbb, obb, out):
    from contextlib import ExitStack
    from concourse.masks import make_identity

    Exp = mybir.ActivationFunctionType.Exp
    f8 = mybir.dt.float8e4
    DR = mybir.MatmulPerfMode.DoubleRow

    with ExitStack() as ctx:
        persist = ctx.enter_context(tc.tile_pool(name="persist", bufs=1))
        wpool = ctx.enter_context(tc.tile_pool(name="wpool", bufs=32))
        ktpool = ctx.enter_context(tc.tile_pool(name="ktpool", bufs=2))
        ktdrpool = ctx.enter_context(tc.tile_pool(name="ktdrpool", bufs=3))
        qtdrpool = ctx.enter_context(tc.tile_pool(name="qtdrpool", bufs=3))
        expool = ctx.enter_context(tc.tile_pool(name="expool", bufs=6))
        rpool = ctx.enter_context(tc.tile_pool(name="rpool", bufs=2))
        opool = ctx.enter_context(tc.tile_pool(name="opool", bufs=5))

        # identity for PE transposes (gpsimd, off the critical engines)
        ident = persist.tile([128, 128], bf16, name="ident", tag="ident")
        make_identity(nc, ident)

        # ---- input DMAs, per-d tiles in consumption order so PE can
        # stream each d-accumulation group behind the DMA arrivals.
        # ts/qw pairwise-interleaved: QT jt0's d-th matmul needs only
        # (ts_d, qw_d), so the first matmul fires ~4us in instead of
        # waiting for all 3MB.  Then kw, llm h0, vw, llm h1, biases, ow.
        qkb_sb = persist.tile([128, 2 * NDT], f32, name="qkb_sb", tag="qkb_sb")
        ts_sb = []
        qw_sb = []
        for d in range(NDT):
            t = persist.tile([128, P], bf16, name=f"ts_sb{d}", tag=f"ts_sb{d}")
            nc.sync.dma_start(out=t, in_=tsT.ap()[d * 128:(d + 1) * 128, :])
            ts_sb.append(t)
            w = wpool.tile([128, D], bf16, name=f"qw_sb{d}", tag="w")
            nc.sync.dma_start(out=w, in_=qwT.ap()[d * 128:(d + 1) * 128, :])
            qw_sb.append(w)
            if d == 0:
                # qkb after the first ts/qw pair: keeps HWDGE slot #1 for
                # the first matmul's inputs, still far ahead of the bias adds
                nc.sync.dma_start(out=qkb_sb, in_=qkb.ap())

        def load_w(dram, prefix):
            tiles = []
            for d in range(NDT):
                t = wpool.tile([128, D], bf16, name=f"{prefix}{d}", tag="w")
                nc.sync.dma_start(out=t, in_=dram.ap()[d * 128:(d + 1) * 128, :])
                tiles.append(t)
            return tiles

        kw_sb = load_w(kwT, "kw_sb")
        # llm as 16 half-tiles [128, 1024]: llm_sb[d][h] covers s-cols
        # h*1024 .. h*1024+1024.  h0 first (KT sc0/sc1 + V' st<8), then vw,
        # then h1 -- matches PE consumption order.
        llm_sb = [[None, None] for _ in range(NDT)]
        for d in range(NDT):
            t = persist.tile([128, 1024], bf16, name=f"llm_sb{d}h0",
                             tag=f"llm_sb{d}h0")
            nc.sync.dma_start(out=t, in_=llmT.ap()[d * 128:(d + 1) * 128,
                                                   0:1024])
            llm_sb[d][0] = t
        vbb_sb = persist.tile([128, D], bf16, name="vbb_sb", tag="vbb_sb")
        nc.sync.dma_start(out=vbb_sb, in_=vbb.ap())
        vw_sb = load_w(vwT, "vw_sb")
        for d in range(NDT):
            t = persist.tile([128, 1024], bf16, name=f"llm_sb{d}h1",
                             tag=f"llm_sb{d}h1")
            nc.sync.dma_start(out=t, in_=llmT.ap()[d * 128:(d + 1) * 128,
                                                   1024:2048])
            llm_sb[d][1] = t
        obb_sb = persist.tile([128, D], bf16, name="obb_sb", tag="obb_sb")
        nc.sync.dma_start(out=obb_sb, in_=obb.ap())
        ow_sb = load_w(owT, "ow_sb")

        # ---- persistent intermediates ----
        qt_sb = []
        for jt in range(NDT):
            qt_sb.append(persist.tile([128, P], f8, name=f"qt_sb{jt}",
                                      tag=f"qt_sb{jt}"))
        qt_dr = [None] * NDT
        kt_sb = [None] * NDT
        kt_dr = [None] * NDT
        vp_sb = [None] * NST
        # ctx_nat: [p(128), pt(4) x d(1024)] bf16 -- normalized context in
        # natural layout, all four p-tiles side by side.
        ctx_nat = persist.tile([128, NPT * D], bf16, name="ctx_nat",
                               tag="ctx_nat")
        cxT = []
        for d in range(NDT):
            cxT.append(persist.tile([128, P], bf16, name=f"cxT{d}",
                                    tag=f"cxT{d}"))
        partial = []
        for T in range(8):
            partial.append(persist.tile([128, 512], bf16, name=f"opart{T}",
                                        tag=f"opart{T}"))

        with tc.tile_pool(name="psS", bufs=2, space="PSUM") as psS, \
             tc.tile_pool(name="psC", bufs=2, space="PSUM") as psC, \
             tc.tile_pool(name="psP", bufs=2, space="PSUM") as psP:

            # ---------------- emission helpers ----------------
            def emit_qt(jt):
                ps = psP.tile([128, P], f32, name=f"ps_q{jt}", tag="psP")
                for d in range(NDT):
                    nc.tensor.matmul(
                        ps,
                        lhsT=qw_sb[d][:, jt * 128:(jt + 1) * 128],
                        rhs=ts_sb[d],
                        start=(d == 0), stop=(d == NDT - 1))
                nc.vector.tensor_scalar_add(qt_sb[jt], ps,
                                            qkb_sb[:, jt:jt + 1])

            def emit_dr_repack(jt, lo, hi):
                # kt_dr[u*32+p, i, s] = kt[u*64+i*32+p, s] (fp8, partition
                # remap via SBUF->SBUF DMA on the idle Pool/SWDGE queue);
                # same for qt_dr when lo==0 and hi==S is not required.
                k3 = kt_dr[jt].rearrange("q (i s) -> q i s", i=2)
                for u in range(2):
                    for i in range(2):
                        nc.gpsimd.dma_start(
                            out=k3[u * 32:(u + 1) * 32, i:i + 1, lo:hi],
                            in_=kt_sb[jt][u * 64 + i * 32:u * 64 + i * 32 + 32,
                                          lo:hi])

            def emit_qt_repack(jt):
                if qt_dr[jt] is None:
                    qt_dr[jt] = qtdrpool.tile([64, 2 * P], f8,
                                              name=f"qt_dr{jt}", tag="qtdr")
                q3 = qt_dr[jt].rearrange("q (i x) -> q i x", i=2)
                for u in range(2):
                    for i in range(2):
                        nc.gpsimd.dma_start(
                            out=q3[u * 32:(u + 1) * 32, i:i + 1, :],
                            in_=qt_sb[jt][u * 64 + i * 32:
                                          u * 64 + i * 32 + 32, :])

            def emit_kt_sc(jt, sc):
                # KT[j, s] for one 512-col s-chunk; llm half h = sc//2.
                if kt_sb[jt] is None:
                    kt_sb[jt] = ktpool.tile([128, S], f8,
                                            name=f"kt_sb{jt}", tag="kt")
                    kt_dr[jt] = ktdrpool.tile([64, 2 * S], f8,
                                              name=f"kt_dr{jt}", tag="ktdr")
                h, c = sc // 2, sc % 2
                ps = psP.tile([128, 512], f32, name=f"ps_k{jt}_{sc}",
                              tag="psP")
                for d in range(NDT):
                    nc.tensor.matmul(
                        ps,
                        lhsT=kw_sb[d][:, jt * 128:(jt + 1) * 128],
                        rhs=llm_sb[d][h][:, c * 512:(c + 1) * 512],
                        start=(d == 0), stop=(d == NDT - 1))
                nc.vector.tensor_scalar_add(
                    kt_sb[jt][:, sc * 512:(sc + 1) * 512], ps,
                    qkb_sb[:, NDT + jt:NDT + jt + 1])
                if jt == 0 and sc >= 2:
                    # sc0/sc1 are consumed by pair-0 stages k<4 via the
                    # non-DR path below, so their repacks are skipped.
                    emit_dr_repack(0, sc * 512, (sc + 1) * 512)
                elif jt >= 1 and sc == 3:
                    emit_dr_repack(jt, 0, S)

            def emit_v(st):
                # V'[s, h*65 + x]: x<64 -> v_h columns, x=64 -> ones
                vp = persist.tile([128, H * (DH + 1)], bf16,
                                  name=f"vp_sb{st}", tag=f"vp_sb{st}")
                vp3 = vp.rearrange("p (h x) -> p h x", x=DH + 1)
                nc.vector.memset(vp3[:, :, DH:DH + 1], 1.0)
                h, c = st // 8, st % 8
                for jc in range(2):
                    ps = psP.tile([128, 512], f32, name=f"ps_v{st}_{jc}",
                                  tag="psP")
                    for d in range(NDT):
                        nc.tensor.matmul(
                            ps,
                            lhsT=llm_sb[d][h][:, c * 128:(c + 1) * 128],
                            rhs=vw_sb[d][:, jc * 512:(jc + 1) * 512],
                            start=(d == 0), stop=(d == NDT - 1))
                    nc.vector.tensor_add(
                        vp3[:, jc * 8:(jc + 1) * 8, 0:DH],
                        ps.rearrange("p (h x) -> p h x", x=DH),
                        vbb_sb[:, jc * 512:(jc + 1) * 512]
                        .rearrange("p (h x) -> p h x", x=DH))
                vp_sb[st] = vp

            emitted_v = [0]

            def ensure_v(upto):
                while emitted_v[0] <= upto:
                    emit_v(emitted_v[0])
                    emitted_v[0] += 1

            def emit_ctx(p, k, ets):
                # ctx[p-chunk, 0:65] += expT_h(st).T @ V'_h, out free = 65.
                # PSUM zero-region semantics: start=True on the FIRST matmul
                # of each psc bank marks the whole 2KB region pending-zero;
                # later chunks' first writes replace-if-pending, so only
                # (st==0, pc==0) starts and only (st==15, pc==3) stops.
                for i in range(2):
                    st = 2 * k + i
                    for u in range(2):
                        h = 2 * p + u
                        for pc in range(NPT):
                            nc.tensor.matmul(
                                psc[u][:, pc * (DH + 1):
                                       (pc + 1) * (DH + 1)],
                                lhsT=ets[u][:, i * 512 + pc * 128:
                                            i * 512 + (pc + 1) * 128],
                                rhs=vp_sb[st][:, h * (DH + 1):
                                              (h + 1) * (DH + 1)],
                                start=(st == 0 and pc == 0),
                                stop=(st == NST - 1 and pc == NPT - 1))

            def emit_normalize(p):
                # On the last pair the Act engine is done with exps, so half
                # the muls go there to shorten the tail's serial chain.
                split = (p == NPAIR - 1)
                rcs = []
                for u in range(2):
                    h = 2 * p + u
                    psc3 = psc[u].rearrange("p (c x) -> p c x", x=DH + 1)
                    rc = rpool.tile([128, NPT], f32, name=f"rc{h}", tag="rc")
                    rc3 = rc.rearrange("p (c x) -> p c x", x=1)
                    nc.vector.reciprocal(rc3, psc3[:, :, DH:DH + 1])
                    rcs.append(rc)
                for pc in range(NPT):
                    for u in range(2):
                        h = 2 * p + u
                        dst = ctx_nat[:, pc * D + h * DH:pc * D + (h + 1) * DH]
                        srcp = psc[u][:, pc * (DH + 1):pc * (DH + 1) + DH]
                        if split and u == 1:
                            nc.scalar.mul(dst, srcp, rcs[u][:, pc:pc + 1])
                        else:
                            nc.vector.tensor_scalar_mul(
                                dst, srcp, rcs[u][:, pc:pc + 1])

            def emit_transposes(p):
                # cxT[p][d, p_global] from ctx_nat pair-p column blocks.
                for pc in range(NPT):
                    pst = psP.tile([128, 128], bf16, name=f"pst{p}_{pc}",
                                   tag="psP")
                    nc.tensor.transpose(
                        pst,
                        ctx_nat[:, pc * D + p * 128:pc * D + (p + 1) * 128],
                        ident)
                    nc.vector.tensor_copy(cxT[p][:, pc * 128:(pc + 1) * 128],
                                          pst)

            def emit_opartial(T):
                # out tile T=(pt,jc): sum d=0..6 plus output bias -> bf16.
                pt, jc = T // 2, T % 2
                ps = psP.tile([128, 512], f32, name=f"ps_op{T}", tag="psP")
                for d in range(NDT - 2):
                    nc.tensor.matmul(
                        ps, lhsT=cxT[d][:, pt * 128:(pt + 1) * 128],
                        rhs=ow_sb[d][:, jc * 512:(jc + 1) * 512],
                        start=(d == 0), stop=(d == NDT - 3))
                nc.vector.tensor_add(partial[T], ps,
                                     obb_sb[:, jc * 512:(jc + 1) * 512])

            # ---------------- phase A ----------------
            for jt in range(NDT):
                emit_qt(jt)
            emit_qt_repack(0)
            emit_kt_sc(0, 0)
            emit_kt_sc(0, 1)

            # ---------------- pipelined head pairs ----------------
            # Stage (p, k) covers s-tiles 2k, 2k+1 of pair p.  ctx lags one
            # stage so PE's in-order queue never parks on the Act engine.
            psc = None
            pend = []  # queue of (p, k, ets); ctx lags scores/exp by 2
            for p in range(NPAIR):
                new_psc = [psC.tile([128, NPT * (DH + 1)], f32,
                                    name=f"psc{2*p+u}", tag="psC")
                           for u in range(2)]
                if p == 0:
                    psc = new_psc
                for k in range(8):
                    pss = [psS.tile([128, 1024], f32,
                                    name=f"ps_s{2*p+u}_{k}", tag="psS")
                           for u in range(2)]
                    if p == 0 and k < 4:
                        # pair-0's first stages outrun the serialized Pool
                        # repack chain; plain fp8 matmuls (1.0 cyc/row) read
                        # the natural layout directly, same numerics.
                        for u in range(2):
                            rs = u * DH
                            for i in range(2):
                                st = 2 * k + i
                                nc.tensor.matmul(
                                    pss[u][:, i * 512:(i + 1) * 512],
                                    lhsT=kt_sb[0][rs:rs + DH,
                                                  st * 128:(st + 1) * 128],
                                    rhs=qt_sb[0][rs:rs + DH, :],
                                    start=True, stop=True)
                    else:
                        k3 = kt_dr[p].rearrange("q (i s) -> q i s", i=2)
                        q3 = qt_dr[p].rearrange("q (i x) -> q i x", i=2)
                        # u-major so pss[u0] is complete after two matmuls
                        # and its exp can issue while u1's scores run.
                        for u in range(2):
                            for i in range(2):
                                st = 2 * k + i
                                nc.tensor.matmul(
                                    pss[u][:, i * 512:(i + 1) * 512],
                                    lhsT=k3[u * 32:(u + 1) * 32, :,
                                            st * 128:(st + 1) * 128],
                                    rhs=q3[u * 32:(u + 1) * 32, :, :],
                                    start=True, stop=True, perf_mode=DR)
                    ets = []
                    for u in range(2):
                        et = expool.tile([128, 1024], bf16,
                                         name=f"et{2*p+u}_{k}", tag="et")
                        nc.scalar.activation(et, pss[u], Exp,
                                             bias=0.0, scale=0.125)
                        ets.append(et)

                    # ---- PE filler for this stage ----
                    if p == 0:
                        if k == 0:
                            emit_qt_repack(1)
                        elif k == 3:
                            emit_kt_sc(0, 2)
                        elif k == 4:
                            emit_kt_sc(1, 0)
                        elif k == 5:
                            emit_kt_sc(0, 3)
                        elif k == 6:
                            emit_kt_sc(1, 1)
                            emit_qt_repack(2)
                        elif k == 7:
                            emit_kt_sc(1, 2)
                            emit_kt_sc(1, 3)
                        ensure_v(2 * k + 1)
                    elif p < NPAIR - 1:
                        if k % 2 == 0:
                            emit_kt_sc(p + 1, k // 2)
                        elif k == 5 and p < NPAIR - 2:
                            emit_qt_repack(p + 2)

                    else:
                        # O-partials T2..T6 at k=3..7 (T0/T1 follow the
                        # transposes(6) in the k==2 post-ctx block);
                        # T7 is held back to cover the tail's exp(7,7) wait.
                        if 3 <= k <= 7:
                            emit_opartial(k - 1)

                    # ---- lagged ctx for the previous stage ----
                    if len(pend) == 2:
                        cp, ck, cets = pend.pop(0)
                        emit_ctx(cp, ck, cets)
                        if ck == 7:
                            emit_normalize(cp)
                            psc = new_psc
                    # transposes of the previous pair go one stage after its
                    # normalize so PE is not queued behind the DVE muls.
                    if k == 2 and p >= 1:
                        emit_transposes(p - 1)
                        if p == NPAIR - 1:
                            emit_opartial(0)
                            emit_opartial(1)
                    pend.append((p, k, ets))

            # ---------------- tail ----------------
            # T7 partial covers PE while Act finishes exp(7,7); then pair-7
            # normalize -> per-p-chunk: transpose, copy, d7 matmul with the
            # bf16 partial accumulated in-PSUM via an identity matmul (PE,
            # 213ns, replaces a 658ns DVE add), Act-engine copy to SBUF
            # (Act is idle in the tail), DMA out.
            cp, ck, cets = pend.pop(0)
            emit_ctx(cp, ck, cets)
            # T7's d-loop is split around ctx(7,7)/normalize so PE covers
            # both the exp(7,7) wait and the normalize-DVE wait.
            ps7 = psP.tile([128, 512], f32, name="ps_op7", tag="psP")
            for d in range(4):
                nc.tensor.matmul(
                    ps7, lhsT=cxT[d][:, 3 * 128:(3 + 1) * 128],
                    rhs=ow_sb[d][:, 512:1024],
                    start=(d == 0), stop=False)
            cp, ck, cets = pend.pop(0)
            emit_ctx(cp, ck, cets)
            for d in range(4, NDT - 2):
                nc.tensor.matmul(
                    ps7, lhsT=cxT[d][:, 3 * 128:(3 + 1) * 128],
                    rhs=ow_sb[d][:, 512:1024],
                    start=False, stop=(d == NDT - 3))
            emit_normalize(NPAIR - 1)
            nc.vector.tensor_add(partial[7], ps7, obb_sb[:, 512:1024])
            pp = NPAIR - 1
            for pc in range(NPT):
                pst = psP.tile([128, 128], bf16, name=f"pst{pp}_{pc}",
                               tag="psP")
                nc.tensor.transpose(
                    pst,
                    ctx_nat[:, pc * D + pp * 128:pc * D + (pp + 1) * 128],
                    ident)
                if pc % 2 == 0:
                    nc.vector.tensor_copy(
                        cxT[pp][:, pc * 128:(pc + 1) * 128], pst)
                else:
                    nc.scalar.copy(cxT[pp][:, pc * 128:(pc + 1) * 128], pst)
            for pc in range(NPT):
                ot = opool.tile([128, 1024], bf16, name=f"ot{pc}", tag="ot",
                                bufs=3)
                for jc in range(2):
                    T = pc * 2 + jc
                    tpool, ttag = ((psS, "psS") if jc == 0 else (psC, "psC"))
                    ps = tpool.tile([128, 512], f32, name=f"ps_o7_{T}",
                                    tag=ttag)
                    for d in (NDT - 2, NDT - 1):
                        nc.tensor.matmul(
                            ps, lhsT=cxT[d][:, pc * 128:(pc + 1) * 128],
                            rhs=ow_sb[d][:, jc * 512:(jc + 1) * 512],
                            start=(d == NDT - 2), stop=False)
                    nc.tensor.matmul(ps, lhsT=ident, rhs=partial[T],
                                     start=False, stop=True)
                    if jc == 0:
                        nc.scalar.copy(ot[:, 0:512], ps)
                    else:
                        nc.vector.tensor_copy(ot[:, 512:1024], ps)
                # one DMA per p-chunk halves the serialized HWDGE issues
                nc.sync.dma_start(
                    out=out.ap()[pc * 128:(pc + 1) * 128, :], in_=ot)


def get_nc():
    global _cached_nc
    if _cached_nc is None:
        _cached_nc = _build_nc()
    return _cached_nc


def make_in_maps(ts_features, llm_features, q_w, q_b, k_w, k_b, v_w, v_b,
                 o_w, o_b):
    ts = np.asarray(ts_features, np.float32)
    llm = np.asarray(llm_features, np.float32)
    shared = {
        "qwT": np.ascontiguousarray(np.asarray(q_w, np.float32).T).astype(_BF16),
        "kwT": np.ascontiguousarray(np.asarray(k_w, np.float32).T).astype(_BF16),
        "vwT": np.ascontiguousarray(np.asarray(v_w, np.float32).T).astype(_BF16),
        "owT": np.ascontiguousarray(np.asarray(o_w, np.float32).T).astype(_BF16),
        "qkb": np.ascontiguousarray(np.concatenate(
            [np.asarray(q_b, np.float32).reshape(NDT, 128).T,
             np.asarray(k_b, np.float32).reshape(NDT, 128).T], axis=1)),
        "vbb": np.ascontiguousarray(
            np.broadcast_to(np.asarray(v_b, np.float32), (128, D))).astype(_BF16),
        "obb": np.ascontiguousarray(
            np.broadcast_to(np.asarray(o_b, np.float32), (128, D))).astype(_BF16),
    }
    in_maps = []
    for b in range(NCORES):
        m = dict(shared)
        m["tsT"] = np.ascontiguousarray(ts[b].T).astype(_BF16)
        m["llmT"] = np.ascontiguousarray(llm[b].T).astype(_BF16)
        in_maps.append(m)
    return in_maps


def kernel(**inputs):
    from concourse.bass_utils import run_bass_kernel_spmd

    nc = get_nc()
    in_maps = make_in_maps(**inputs)
    res = run_bass_kernel_spmd(nc, in_maps, list(range(NCORES)))
    return np.stack([res.results[i]["out"] for i in range(NCORES)],
                    axis=0).astype(np.float32)
